# revision 38
# baseline (speedup 1.0000x reference)
"""Multi-head attention (B=8, N=1024, C=768, H=12, D=64) on 8 TRN2 NeuronCores.

Sharding: pure data parallelism - one batch element per core, no collectives.

v5: v2's software pipeline (ScalarE exp paced, PE filled with qkv/proj work)
plus:
  - host pre-tiles every input into its exact SBUF layout so each DMA is a
    single [128 x contiguous] transfer (x split per-ci so the first qk
    accumulation chain starts as chunks land);
  - softmax exp runs as one ACTIVATE per TWO m-tiles ([128,2048] over a
    fixed 4-bank score tile) halving ScalarE instruction overhead;
  - div_finish for block i-1 is emitted before block i's AV loop and the
    reciprocal chain is batched as [1,1024];
  - e_block(0) feeds the last block's filler slots so the output projection
    overlaps the final AV/exp instead of trailing it.

Per-core dataflow (matmuls bf16, accumulation fp32 in PSUM):
  b(hp): qT, kT [d, n] for head pair hp (head-dim on partitions).
  C_a/C_b: v natural [tokens, h*65], 65th col per head = 1.0 (denominator).
  Block (hp, nn): per mt-pair, 4 score matmuls (row-group concurrent per
    head) land in st_big [128,2048] -> one 2048-wide exp on ScalarE (scale
    fused) -> bf16 pt pair; AV chains for both heads interleaved with the
    NEXT block's score pairs; denominator lands in row 64 of the AV psum;
    DVE reciprocal -> ones2-matmul broadcast -> DVE multiply writes
    normalized oT.
  E: yT[o, n] = pwT.T @ oT + bias.
Host transposes yT back to [N, C].
"""

import numpy as np

B, N, C, H, D = 8, 1024, 768, 12, 64
SCALE = D ** -0.5
NCORES = 8

CT = C // 128   # 6  c-tiles
HP = H // 2     # 6  head pairs (2 heads of 64 share a 128-partition tile)
NT = N // 512   # 2  n-chunks of 512
MT = N // 128   # 8  m-tiles (keys)
MP = MT // 2    # 4  m-tile pairs (exp granularity)
VW = 65         # v columns per head (64 data + 1 ones)

_CACHE = {}


def _build_nc(dbg=False):
    import concourse.bass as bass
    import concourse.mybir as mybir
    import concourse.tile as tile
    from concourse import bacc

    f32 = mybir.dt.float32
    bf16 = mybir.dt.bfloat16
    AF = mybir.ActivationFunctionType

    nc = bacc.Bacc(
        "TRN2",
        target_bir_lowering=False,
        debug=False,
        enable_asserts=False,
        num_devices=NCORES,
    )

    # all inputs pre-tiled on host to the exact SBUF layout -> each DMA is
    # [128 partitions x contiguous bytes]
    xT_d = nc.dram_tensor("xT", [128, CT * N], bf16, kind="ExternalInput").ap()
    wq_d = nc.dram_tensor("wqT", [128, HP * CT * 128], bf16,
                          kind="ExternalInput").ap()
    wk_d = nc.dram_tensor("wkT", [128, HP * CT * 128], bf16,
                          kind="ExternalInput").ap()
    wv_d = nc.dram_tensor("wvT", [128, CT * C], bf16, kind="ExternalInput").ap()
    pw_d = nc.dram_tensor("pwT", [128, CT * C], bf16, kind="ExternalInput").ap()
    ones_d = nc.dram_tensor("ones2", [2, 128], bf16, kind="ExternalInput").ap()
    pb_d = nc.dram_tensor("pb", [128, CT], f32, kind="ExternalInput").ap()
    out_d = nc.dram_tensor("out", [C, N], bf16, kind="ExternalOutput").ap()

    with tile.TileContext(nc) as tc:
        data = tc.alloc_tile_pool(name="data", bufs=1)
        psp = tc.alloc_tile_pool(name="psp", bufs=1, space="PSUM")
        ptp = data
        small = data

        pb_sb = data.tile([128, CT], f32)
        ones2 = data.tile([2, 128], bf16)

        xTs = data.tile([128, CT * N], bf16)

        # wq/wk hp-major in sbuf: cols = hp*768 + ci*128 + j
        wqs = data.tile([128, HP * CT * 128], bf16)
        wks = data.tile([128, HP * CT * 128], bf16)
        wvs = data.tile([128, CT * C], bf16)
        pws = data.tile([128, CT * C], bf16)

        # dummy tile for PE warm-up (memset first: it gates the warm-up)
        wdum = data.tile([128, 640], bf16)
        nc.gpsimd.memset(wdum[:], 0.0)

        # priority order: what the bootstrap needs first; x per-ci, spread
        # across three engine DMA queues so transfers overlap.
        nc.sync.dma_start(xTs[:, 0:N], xT_d[:, 0:N])               # x ci=0
        nc.sync.dma_start(wqs[:, 0:768], wq_d[:, 0:768])           # wq hp0
        nc.sync.dma_start(wks[:, 0:768], wk_d[:, 0:768])           # wk hp0
        for ci in (1, 2):
            nc.sync.dma_start(xTs[:, ci * N:(ci + 1) * N],
                              xT_d[:, ci * N:(ci + 1) * N])
        for ci in (3, 4, 5):
            nc.gpsimd.dma_start(xTs[:, ci * N:(ci + 1) * N],
                                xT_d[:, ci * N:(ci + 1) * N])
        nc.scalar.dma_start(wks[:, 768:HP * 768], wk_d[:, 768:HP * 768])
        nc.scalar.dma_start(wqs[:, 768:HP * 768], wq_d[:, 768:HP * 768])
        nc.scalar.dma_start(wvs[:], wv_d[:])
        nc.scalar.dma_start(pws[:], pw_d[:])
        nc.scalar.dma_start(ones2[:], ones_d[:])
        nc.scalar.dma_start(pb_sb[:], pb_d[:])

        qT = data.tile([128, HP * N], bf16)
        kT = data.tile([128, HP * N], bf16)
        va = data.tile([128, MT * H * VW], bf16)
        oT = data.tile([128, HP * N], bf16)

        # ones columns of v (softmax denominator trick)
        v3 = va[:].rearrange("p (x e) -> p x e", e=VW)
        nc.gpsimd.memset(v3[:, :, 64:65], 1.0)

        def qk_chain(dst, w, hp, nn):
            """One 6-matmul accumulation chain + cast for q or k."""
            ps = psp.tile([128, 512], f32, tag="out", bufs=2, name="ps_qk")
            for ci in range(CT):
                nc.tensor.matmul(
                    ps[:],
                    w[:, hp * 768 + ci * 128: hp * 768 + ci * 128 + 128],
                    xTs[:, ci * N + nn * 512: ci * N + nn * 512 + 512],
                    start=(ci == 0), stop=(ci == CT - 1),
                )
                if ci < CT - 1:
                    yield
            nc.vector.tensor_copy(
                dst[:, hp * N + nn * 512: hp * N + nn * 512 + 512], ps[:])
            yield

        def gen_b_block(hp):
            for dst, w in ((qT, wqs), (kT, wks)):
                for nn in range(NT):
                    yield from qk_chain(dst, w, hp, nn)

        def gen_b5():
            # hp5 in scores-consumption order: q(5,0)+k(5,0) gate block 9's
            # first scores, k(5,1) gates mt>=4, q(5,1) gates block 10 - so
            # the last two chains can drip into block 9's filler slots.
            yield from qk_chain(qT, wqs, 5, 0)
            yield from qk_chain(kT, wks, 5, 0)
            yield from qk_chain(kT, wks, 5, 1)
            yield from qk_chain(qT, wqs, 5, 1)

        def qk0_pair():
            """Interleaved q(0,0)/k(0,0) chains: each x chunk feeds two
            matmuls as it lands, keeping the PE dense during the DMA-paced
            startup."""
            psq = psp.tile([128, 512], f32, tag="out", bufs=2, name="ps_q0")
            psk = psp.tile([128, 512], f32, tag="out", bufs=2, name="ps_k0")
            for ci in range(CT):
                nc.tensor.matmul(
                    psq[:], wqs[:, ci * 128: ci * 128 + 128],
                    xTs[:, ci * N: ci * N + 512],
                    start=(ci == 0), stop=(ci == CT - 1))
                nc.tensor.matmul(
                    psk[:], wks[:, ci * 128: ci * 128 + 128],
                    xTs[:, ci * N: ci * N + 512],
                    start=(ci == 0), stop=(ci == CT - 1))
            nc.vector.tensor_copy(qT[:, 0:512], psq[:])
            nc.vector.tensor_copy(kT[:, 0:512], psk[:])

        def c_chain(oc, mt):
            """One v-projection chain + cast for m-chunk mt, out-half oc."""
            ow = 512 if oc == 0 else 256
            nh = ow // 64
            ps = psp.tile([128, 512], f32, tag="out", bufs=2, name="ps_v")
            for ci in range(CT):
                nc.tensor.matmul(
                    ps[:, :ow],
                    xTs[:, ci * N + mt * 128: ci * N + mt * 128 + 128],
                    wvs[:, ci * C + oc * 512: ci * C + oc * 512 + ow],
                    start=(ci == 0), stop=(ci == CT - 1),
                )
                if ci < CT - 1:
                    yield
            dst3 = va[:, mt * H * VW:(mt + 1) * H * VW].rearrange(
                "p (h e) -> p h e", e=VW)[:, oc * 8: oc * 8 + nh, 0:64]
            src3 = ps[:, :ow].rearrange("p (h d) -> p h d", d=64)
            nc.vector.tensor_copy(dst3, src3)
            yield

        def gen_c_block(oc, mt0, mt1):
            for mt in range(mt0, mt1):
                yield from c_chain(oc, mt)

        def s_block_step(hp, nn, mt, pts):
            """Score pair (both heads of hp) for m-chunk mt, exp into a
            per-mt pt tile (exact dependency granularity for AV)."""
            st = psp.tile([128, 1024], f32, tag="st", bufs=2, name="st")
            for hi in range(2):
                lo = 64 * hi
                nc.tensor.matmul(
                    st[:, hi * 512:(hi + 1) * 512],
                    kT[lo:lo + 64,
                       hp * N + mt * 128: hp * N + mt * 128 + 128],
                    qT[lo:lo + 64,
                       hp * N + nn * 512: hp * N + nn * 512 + 512],
                )
            pt = ptp.tile([128, 1024], bf16, tag="pt", bufs=16, name="pt")
            nc.scalar.activation(
                pt[:].rearrange("p (g x) -> p g x", g=2),
                st[:].rearrange("p (g x) -> p g x", g=2),
                AF.Exp, scale=SCALE)
            pts.append(pt)

        def av_step(av, hp, hi, mt, pts):
            h = 2 * hp + hi
            nc.tensor.matmul(
                av[0:VW, :],
                va[:, mt * H * VW + h * VW: mt * H * VW + h * VW + VW],
                pts[mt][:, hi * 512: hi * 512 + 512],
                start=(mt == 0), stop=(mt == MT - 1),
            )

        def div_start(hp, nn, avs):
            """Right after a block's AV chains: copy raw outputs to SBUF
            (frees the av psum slots asap) + reciprocals of the two
            denominator rows batched as [1,1024]. reciprocal_approx_fast
            needs SBUF input."""
            oraw = small.tile([128, 512], bf16, tag="oraw", bufs=6, name="oraw")
            nc.vector.tensor_copy(oraw[0:64, :], avs[0][0:64, :])
            nc.vector.tensor_copy(oraw[64:128, :], avs[1][0:64, :])
            d32 = small.tile([1, 1024], f32, tag="den32", bufs=6, name="den32")
            nc.vector.tensor_copy(d32[:, 0:512], avs[0][64:65, :])
            nc.vector.tensor_copy(d32[:, 512:1024], avs[1][64:65, :])
            r32 = small.tile([1, 1024], f32, tag="recip32", bufs=6, name="recip32")
            nc.vector.reciprocal_approx_fast(r32[:], d32[:])
            r2 = small.tile([1, 1024], bf16, tag="recip2", bufs=6, name="recip2")
            nc.vector.tensor_copy(r2[:], r32[:])
            rs = [r2[:, 0:512], r2[:, 512:1024]]
            return (hp, nn, oraw, rs)

        def div_finish(st):
            """Deferred one block: broadcast recips to partition halves via
            ones-matmuls, then normalize into oT (inputs long ready, so the
            bc matmuls never stall the tensor queue)."""
            hp, nn, oraw, rs = st
            blk = slice(hp * N + nn * 512, hp * N + nn * 512 + 512)
            bc = psp.tile([128, 512], f32, tag="out", bufs=2, name="bc")
            nc.tensor.matmul(bc[0:64, :], ones2[0:1, 0:64], rs[0],
                             start=True, stop=True)
            nc.tensor.matmul(bc[64:128, :], ones2[0:1, 0:64], rs[1],
                             start=True, stop=True)
            nc.vector.tensor_mul(oT[:, blk], oraw[:], bc[:])

        def e_chain(nn, ot):
            """One output-projection chain for (n-chunk, out-tile)."""
            yp = psp.tile([128, 512], f32, tag="out", bufs=2, name="yp")
            for ci in range(CT):
                nc.tensor.matmul(
                    yp[:],
                    pws[:, ci * C + ot * 128: ci * C + ot * 128 + 128],
                    oT[:, ci * N + nn * 512: ci * N + nn * 512 + 512],
                    start=(ci == 0), stop=(ci == CT - 1),
                )
                if ci < CT - 1:
                    yield
            ys = small.tile([128, 512], bf16, tag="ys", bufs=6, name="ys")
            nc.vector.tensor_scalar_add(ys[:], yp[:], pb_sb[:, ot:ot + 1])
            nc.sync.dma_start(
                out_d[ot * 128:(ot + 1) * 128, nn * 512:(nn + 1) * 512],
                ys[:])
            yield

        def gen_e_block(nn):
            for ot in range(CT):
                yield from e_chain(nn, ot)

        # ---------------- pipelined emission ----------------
        blocks = [(hp, nn) for hp in range(HP) for nn in range(NT)]

        # filler: remaining qkv/v tensor work, drip-fed into j-loop steps
        # so the PE stays continuously busy while ScalarE paces on exp.
        import itertools
        filler_gen = [itertools.chain(
            gen_b_block(2), gen_b_block(3), gen_b_block(4),
            gen_c_block(1, 0, 8), gen_b5())]
        pulled = [0]

        def pull(k):
            for _ in range(k):
                try:
                    next(filler_gen[0])
                    pulled[0] += 1
                except StopIteration:
                    return

        # filler units that must be emitted before block i's j-loop:
        # b2<=i3 (24), b3<=i5 (48), b4<=i7 (72), C_b<=i8 (120),
        # b5 q50/k50 <= i9 (132); k51/q51 drip into block 9's pulls.
        DEADLINE = {3: 24, 5: 48, 7: 72, 8: 120, 9: 132}

        # ---- PE warm-up: junk matmuls while the input DMAs stream ------
        warm_ps = psp.tile([128, 512], f32, tag="out", bufs=2, name="warm")

        def keep_warm(k):
            for _ in range(k):
                nc.tensor.matmul(warm_ps[:], wdum[:, 0:128], wdum[:, 128:640],
                                 start=True, stop=True)

        keep_warm(10)

        # ---- bootstrap: earliest possible exp start --------------------
        cur_pts = []
        qk0_pair()
        for mt in range(4):
            s_block_step(0, 0, mt, cur_pts)
        for _ in qk_chain(kT, wks, 0, 1):
            pass
        for mt in range(4, MT):
            s_block_step(0, 0, mt, cur_pts)
        for _ in qk_chain(qT, wqs, 0, 1):
            pass
        for _ in gen_b_block(1):
            pass
        for _ in gen_c_block(0, 0, 8):
            pass

        # ---- steady state ----------------------------------------------
        pending = None
        for i, (hp, nn) in enumerate(blocks):
            need = DEADLINE.get(i, 0)
            while pulled[0] < need:
                pull(1)
            nxt = blocks[i + 1] if i + 1 < len(blocks) else None
            nxt_pts = []
            # finish the PREVIOUS block's normalize first: its recips are
            # long done, so the broadcasts + mul overlap this block's AV
            # instead of queueing behind this block's reciprocal chain.
            if pending is not None:
                div_finish(pending)
                pending = None
            if i == 11:
                # all nn0 oT slabs are normalized; the output projection
                # becomes this block's filler so it overlaps the final AV.
                filler_gen[0] = gen_e_block(0)
            av0 = psp.tile([128, 512], f32, tag="av", bufs=2, name="av0")
            av1 = psp.tile([128, 512], f32, tag="av", bufs=2, name="av1")
            for mt in range(MT):
                av_step(av0, hp, 0, mt, cur_pts)
                av_step(av1, hp, 1, mt, cur_pts)
                if nxt:
                    s_block_step(nxt[0], nxt[1], mt, nxt_pts)
                pull(2)
            st_div = div_start(hp, nn, (av0, av1))
            pull(2)
            pending = st_div
            cur_pts = nxt_pts
        for _ in filler_gen[0]:   # drain any remaining e_block(0) work
            pass
        div_finish(pending)
        for _ in gen_e_block(1):
            pass

        psp.release()
        data.release()

    # Keep semaphore waits on the MATMULs instead of migrating them onto
    # their LDWEIGHTS: a wait-carrying LDWEIGHTS cannot be pulled ahead by
    # the PE's reorder window, which costs ~100ns per affected matmul.
    # Extra waits get split into EVENT_SEMAPHORE instructions instead.
    nc.move_matmul_waits_to_ldweights = lambda: None
    nc.compile()
    return nc


def _get_nc():
    if "nc" not in _CACHE:
        _CACHE["nc"] = _build_nc()
    return _CACHE["nc"]


def _prep_in_maps(x, qkv_w, proj_w, proj_b):
    import ml_dtypes

    bf16 = ml_dtypes.bfloat16
    x = np.asarray(x, dtype=np.float32)
    qkv_w = np.asarray(qkv_w, dtype=np.float32)
    proj_w = np.asarray(proj_w, dtype=np.float32)
    proj_b = np.asarray(proj_b, dtype=np.float32)

    def by_hp(wT):  # [C(in), C(out)] -> sbuf layout [128, HP*CT*128]
        w4 = wT.reshape(CT, 128, HP, 128).transpose(1, 2, 0, 3)
        return np.ascontiguousarray(w4.reshape(128, HP * CT * 128)).astype(bf16)

    def by_ci(wT):  # [C(in), C(out)] -> sbuf layout [128, CT*C]
        w3 = wT.reshape(CT, 128, C).transpose(1, 0, 2)
        return np.ascontiguousarray(w3.reshape(128, CT * C)).astype(bf16)

    wq_hp = by_hp(np.ascontiguousarray(qkv_w[0:C].T))         # [in, out] tiled
    wk_hp = by_hp(np.ascontiguousarray(qkv_w[C:2 * C].T))
    wvT = by_ci(np.ascontiguousarray(qkv_w[2 * C:3 * C].T))
    pwT = by_ci(np.ascontiguousarray(proj_w.T))
    pb = np.ascontiguousarray(proj_b.reshape(CT, 128).T)      # [128, CT] f32
    ones2 = np.zeros((2, 128), dtype=np.float32)
    ones2[0, 0:64] = 1.0
    ones2[1, 64:128] = 1.0
    ones2 = ones2.astype(bf16)

    in_maps = []
    for b in range(B):
        # xT sbuf layout [128, CT*N]: col ci*N+n = x[n, ci*128+p]
        xt = np.ascontiguousarray(
            x[b].T.reshape(CT, 128, N).transpose(1, 0, 2).reshape(128, CT * N)
        ).astype(bf16)
        in_maps.append({
            "xT": xt,
            "wqT": wq_hp, "wkT": wk_hp, "wvT": wvT, "pwT": pwT, "pb": pb,
            "ones2": ones2,
        })
    return in_maps


def _run(in_maps, **kwargs):
    from concourse.bass_utils import run_bass_kernel_spmd

    return run_bass_kernel_spmd(_get_nc(), in_maps,
                                core_ids=list(range(NCORES)), **kwargs)


def _gather(res):
    out = np.stack([res.results[b]["out"].T for b in range(B)], axis=0)
    return np.ascontiguousarray(out.astype(np.float32))


def kernel(x, qkv_w, proj_w, proj_b):
    return _gather(_run(_prep_in_maps(x, qkv_w, proj_w, proj_b)))


# revision 39
# speedup vs baseline: 1.0684x; 1.0684x over previous
"""Multi-head attention (B=8, N=1024, C=768, H=12, D=64) on 8 TRN2 NeuronCores.

Sharding: pure data parallelism - one batch element per core, no collectives.

v5: v2's software pipeline (ScalarE exp paced, PE filled with qkv/proj work)
plus:
  - host pre-tiles every input into its exact SBUF layout so each DMA is a
    single [128 x contiguous] transfer (x split per-ci so the first qk
    accumulation chain starts as chunks land);
  - softmax exp runs as one ACTIVATE per TWO m-tiles ([128,2048] over a
    fixed 4-bank score tile) halving ScalarE instruction overhead;
  - div_finish for block i-1 is emitted before block i's AV loop and the
    reciprocal chain is batched as [1,1024];
  - e_block(0) feeds the last block's filler slots so the output projection
    overlaps the final AV/exp instead of trailing it.

Per-core dataflow (matmuls bf16, accumulation fp32 in PSUM):
  b(hp): qT, kT [d, n] for head pair hp (head-dim on partitions).
  C_a/C_b: v natural [tokens, h*65], 65th col per head = 1.0 (denominator).
  Block (hp, nn): per mt-pair, 4 score matmuls (row-group concurrent per
    head) land in st_big [128,2048] -> one 2048-wide exp on ScalarE (scale
    fused) -> bf16 pt pair; AV chains for both heads interleaved with the
    NEXT block's score pairs; denominator lands in row 64 of the AV psum;
    DVE reciprocal -> ones2-matmul broadcast -> DVE multiply writes
    normalized oT.
  E: yT[o, n] = pwT.T @ oT + bias.
Host transposes yT back to [N, C].
"""

import numpy as np

B, N, C, H, D = 8, 1024, 768, 12, 64
SCALE = D ** -0.5
NCORES = 8

CT = C // 128   # 6  c-tiles
HP = H // 2     # 6  head pairs (2 heads of 64 share a 128-partition tile)
NT = N // 512   # 2  n-chunks of 512
MT = N // 128   # 8  m-tiles (keys)
MP = MT // 2    # 4  m-tile pairs (exp granularity)
VW = 65         # v columns per head (64 data + 1 ones)

_CACHE = {}


def _build_nc(dbg=False):
    import concourse.bass as bass
    import concourse.mybir as mybir
    import concourse.tile as tile
    from concourse import bacc

    f32 = mybir.dt.float32
    bf16 = mybir.dt.bfloat16
    AF = mybir.ActivationFunctionType

    nc = bacc.Bacc(
        "TRN2",
        target_bir_lowering=False,
        debug=False,
        enable_asserts=False,
        num_devices=NCORES,
    )

    # all inputs pre-tiled on host to the exact SBUF layout -> each DMA is
    # [128 partitions x contiguous bytes]
    xT_d = nc.dram_tensor("xT", [128, CT * N], bf16, kind="ExternalInput").ap()
    wq_d = nc.dram_tensor("wqT", [128, HP * CT * 128], bf16,
                          kind="ExternalInput").ap()
    wk_d = nc.dram_tensor("wkT", [128, HP * CT * 128], bf16,
                          kind="ExternalInput").ap()
    wv_d = nc.dram_tensor("wvT", [128, CT * C], bf16, kind="ExternalInput").ap()
    pw_d = nc.dram_tensor("pwT", [128, CT * C], bf16, kind="ExternalInput").ap()
    ones_d = nc.dram_tensor("ones2", [2, 128], bf16, kind="ExternalInput").ap()
    pb_d = nc.dram_tensor("pb", [128, CT], f32, kind="ExternalInput").ap()
    out_d = nc.dram_tensor("out", [C, N], bf16, kind="ExternalOutput").ap()

    with tile.TileContext(nc) as tc:
        data = tc.alloc_tile_pool(name="data", bufs=1)
        psp = tc.alloc_tile_pool(name="psp", bufs=1, space="PSUM")
        ptp = data
        small = data

        pb_sb = data.tile([128, CT], f32)
        ones2 = data.tile([2, 128], bf16)

        xTs = data.tile([128, CT * N], bf16)

        # wq/wk hp-major in sbuf: cols = hp*768 + ci*128 + j
        wqs = data.tile([128, HP * CT * 128], bf16)
        wks = data.tile([128, HP * CT * 128], bf16)
        wvs = data.tile([128, CT * C], bf16)
        pws = data.tile([128, CT * C], bf16)

        # dummy tile for PE warm-up (memset first: it gates the warm-up)
        wdum = data.tile([128, 640], bf16)
        nc.gpsimd.memset(wdum[:], 0.0)

        # priority order: what the bootstrap needs first; x per-ci so the
        # first accumulation chains start while later chunks stream in.
        nc.sync.dma_start(xTs[:, 0:N], xT_d[:, 0:N])               # x ci=0
        nc.sync.dma_start(wqs[:, 0:768], wq_d[:, 0:768])           # wq hp0
        nc.sync.dma_start(wks[:, 0:768], wk_d[:, 0:768])           # wk hp0
        for ci in range(1, CT):
            nc.sync.dma_start(xTs[:, ci * N:(ci + 1) * N],
                              xT_d[:, ci * N:(ci + 1) * N])
        nc.sync.dma_start(wks[:, 768:HP * 768], wk_d[:, 768:HP * 768])
        nc.sync.dma_start(wqs[:, 768:HP * 768], wq_d[:, 768:HP * 768])
        nc.sync.dma_start(wvs[:], wv_d[:])
        nc.sync.dma_start(pws[:], pw_d[:])
        nc.sync.dma_start(ones2[:], ones_d[:])
        nc.sync.dma_start(pb_sb[:], pb_d[:])

        qT = data.tile([128, HP * N], bf16)
        kT = data.tile([128, HP * N], bf16)
        va = data.tile([128, MT * H * VW], bf16)
        oT = data.tile([128, HP * N], bf16)

        # ones columns of v (softmax denominator trick)
        v3 = va[:].rearrange("p (x e) -> p x e", e=VW)
        nc.gpsimd.memset(v3[:, :, 64:65], 1.0)

        def qk_chain(dst, w, hp, nn):
            """One 6-matmul accumulation chain + cast for q or k."""
            ps = psp.tile([128, 512], f32, tag="out", bufs=2, name="ps_qk")
            for ci in range(CT):
                nc.tensor.matmul(
                    ps[:],
                    w[:, hp * 768 + ci * 128: hp * 768 + ci * 128 + 128],
                    xTs[:, ci * N + nn * 512: ci * N + nn * 512 + 512],
                    start=(ci == 0), stop=(ci == CT - 1),
                )
                if ci < CT - 1:
                    yield
            nc.vector.tensor_copy(
                dst[:, hp * N + nn * 512: hp * N + nn * 512 + 512], ps[:])
            yield

        def gen_b_block(hp):
            for dst, w in ((qT, wqs), (kT, wks)):
                for nn in range(NT):
                    yield from qk_chain(dst, w, hp, nn)

        def gen_b5():
            # hp5 in scores-consumption order: q(5,0)+k(5,0) gate block 9's
            # first scores, k(5,1) gates mt>=4, q(5,1) gates block 10 - so
            # the last two chains can drip into block 9's filler slots.
            yield from qk_chain(qT, wqs, 5, 0)
            yield from qk_chain(kT, wks, 5, 0)
            yield from qk_chain(kT, wks, 5, 1)
            yield from qk_chain(qT, wqs, 5, 1)

        def qk0_pair():
            """Interleaved q(0,0)/k(0,0) chains: each x chunk feeds two
            matmuls as it lands, keeping the PE dense during the DMA-paced
            startup."""
            psq = psp.tile([128, 512], f32, tag="out", bufs=2, name="ps_q0")
            psk = psp.tile([128, 512], f32, tag="out", bufs=2, name="ps_k0")
            for ci in range(CT):
                nc.tensor.matmul(
                    psq[:], wqs[:, ci * 128: ci * 128 + 128],
                    xTs[:, ci * N: ci * N + 512],
                    start=(ci == 0), stop=(ci == CT - 1))
                nc.tensor.matmul(
                    psk[:], wks[:, ci * 128: ci * 128 + 128],
                    xTs[:, ci * N: ci * N + 512],
                    start=(ci == 0), stop=(ci == CT - 1))
            nc.vector.tensor_copy(qT[:, 0:512], psq[:])
            nc.vector.tensor_copy(kT[:, 0:512], psk[:])

        def c_chain(oc, mt):
            """One v-projection chain + cast for m-chunk mt, out-half oc."""
            ow = 512 if oc == 0 else 256
            nh = ow // 64
            ps = psp.tile([128, 512], f32, tag="out", bufs=2, name="ps_v")
            for ci in range(CT):
                nc.tensor.matmul(
                    ps[:, :ow],
                    xTs[:, ci * N + mt * 128: ci * N + mt * 128 + 128],
                    wvs[:, ci * C + oc * 512: ci * C + oc * 512 + ow],
                    start=(ci == 0), stop=(ci == CT - 1),
                )
                if ci < CT - 1:
                    yield
            dst3 = va[:, mt * H * VW:(mt + 1) * H * VW].rearrange(
                "p (h e) -> p h e", e=VW)[:, oc * 8: oc * 8 + nh, 0:64]
            src3 = ps[:, :ow].rearrange("p (h d) -> p h d", d=64)
            nc.vector.tensor_copy(dst3, src3)
            yield

        def gen_c_block(oc, mt0, mt1):
            for mt in range(mt0, mt1):
                yield from c_chain(oc, mt)

        def s_block_step(hp, nn, mt, pts):
            """Score pair (both heads of hp) for m-chunk mt, exp into a
            per-mt pt tile (exact dependency granularity for AV)."""
            st = psp.tile([128, 1024], f32, tag="st", bufs=2, name="st")
            for hi in range(2):
                lo = 64 * hi
                nc.tensor.matmul(
                    st[:, hi * 512:(hi + 1) * 512],
                    kT[lo:lo + 64,
                       hp * N + mt * 128: hp * N + mt * 128 + 128],
                    qT[lo:lo + 64,
                       hp * N + nn * 512: hp * N + nn * 512 + 512],
                )
            pt = ptp.tile([128, 1024], bf16, tag="pt", bufs=16, name="pt")
            nc.scalar.activation(
                pt[:].rearrange("p (g x) -> p g x", g=2),
                st[:].rearrange("p (g x) -> p g x", g=2),
                AF.Exp, scale=SCALE)
            pts.append(pt)

        def av_step(av, hp, hi, mt, pts):
            h = 2 * hp + hi
            nc.tensor.matmul(
                av[0:VW, :],
                va[:, mt * H * VW + h * VW: mt * H * VW + h * VW + VW],
                pts[mt][:, hi * 512: hi * 512 + 512],
                start=(mt == 0), stop=(mt == MT - 1),
            )

        def div_start(hp, nn, avs):
            """Right after a block's AV chains: copy raw outputs to SBUF
            (frees the av psum slots asap) + reciprocals of the two
            denominator rows batched as [1,1024]. reciprocal_approx_fast
            needs SBUF input."""
            oraw = small.tile([128, 512], bf16, tag="oraw", bufs=6, name="oraw")
            nc.vector.tensor_copy(oraw[0:64, :], avs[0][0:64, :])
            nc.vector.tensor_copy(oraw[64:128, :], avs[1][0:64, :])
            d32 = small.tile([1, 1024], f32, tag="den32", bufs=6, name="den32")
            nc.vector.tensor_copy(d32[:, 0:512], avs[0][64:65, :])
            nc.vector.tensor_copy(d32[:, 512:1024], avs[1][64:65, :])
            r32 = small.tile([1, 1024], f32, tag="recip32", bufs=6, name="recip32")
            nc.vector.reciprocal_approx_fast(r32[:], d32[:])
            r2 = small.tile([1, 1024], bf16, tag="recip2", bufs=6, name="recip2")
            nc.vector.tensor_copy(r2[:], r32[:])
            rs = [r2[:, 0:512], r2[:, 512:1024]]
            return (hp, nn, oraw, rs)

        def div_finish(st):
            """Deferred one block: broadcast recips to partition halves via
            ones-matmuls, then normalize into oT (inputs long ready, so the
            bc matmuls never stall the tensor queue)."""
            hp, nn, oraw, rs = st
            blk = slice(hp * N + nn * 512, hp * N + nn * 512 + 512)
            bc = psp.tile([128, 512], f32, tag="out", bufs=2, name="bc")
            nc.tensor.matmul(bc[0:64, :], ones2[0:1, 0:64], rs[0],
                             start=True, stop=True)
            nc.tensor.matmul(bc[64:128, :], ones2[0:1, 0:64], rs[1],
                             start=True, stop=True)
            nc.vector.tensor_mul(oT[:, blk], oraw[:], bc[:])

        def e_chain(nn, ot):
            """One output-projection chain for (n-chunk, out-tile)."""
            yp = psp.tile([128, 512], f32, tag="out", bufs=2, name="yp")
            for ci in range(CT):
                nc.tensor.matmul(
                    yp[:],
                    pws[:, ci * C + ot * 128: ci * C + ot * 128 + 128],
                    oT[:, ci * N + nn * 512: ci * N + nn * 512 + 512],
                    start=(ci == 0), stop=(ci == CT - 1),
                )
                if ci < CT - 1:
                    yield
            ys = small.tile([128, 512], bf16, tag="ys", bufs=6, name="ys")
            nc.vector.tensor_scalar_add(ys[:], yp[:], pb_sb[:, ot:ot + 1])
            nc.sync.dma_start(
                out_d[ot * 128:(ot + 1) * 128, nn * 512:(nn + 1) * 512],
                ys[:])
            yield

        def gen_e_block(nn):
            for ot in range(CT):
                yield from e_chain(nn, ot)

        # ---------------- pipelined emission ----------------
        blocks = [(hp, nn) for hp in range(HP) for nn in range(NT)]

        # filler: remaining qkv/v tensor work, drip-fed into j-loop steps
        # so the PE stays continuously busy while ScalarE paces on exp.
        import itertools
        filler_gen = [itertools.chain(
            gen_b_block(2), gen_b_block(3), gen_b_block(4),
            gen_c_block(1, 0, 8), gen_b5())]
        pulled = [0]

        def pull(k):
            for _ in range(k):
                try:
                    next(filler_gen[0])
                    pulled[0] += 1
                except StopIteration:
                    return

        # filler units that must be emitted before block i's j-loop:
        # b2<=i3 (24), b3<=i5 (48), b4<=i7 (72), C_b<=i8 (120),
        # b5 q50/k50 <= i9 (132); k51/q51 drip into block 9's pulls.
        DEADLINE = {3: 24, 5: 48, 7: 72, 8: 120, 9: 132}

        # ---- PE warm-up: junk matmuls while the input DMAs stream ------
        warm_ps = psp.tile([128, 512], f32, tag="out", bufs=2, name="warm")

        def keep_warm(k):
            for _ in range(k):
                nc.tensor.matmul(warm_ps[:], wdum[:, 0:128], wdum[:, 128:640],
                                 start=True, stop=True)

        keep_warm(10)

        # ---- bootstrap: earliest possible exp start --------------------
        cur_pts = []
        qk0_pair()
        for mt in range(4):
            s_block_step(0, 0, mt, cur_pts)
        for _ in qk_chain(kT, wks, 0, 1):
            pass
        for mt in range(4, MT):
            s_block_step(0, 0, mt, cur_pts)
        for _ in qk_chain(qT, wqs, 0, 1):
            pass
        for _ in gen_b_block(1):
            pass
        for _ in gen_c_block(0, 0, 8):
            pass

        # ---- steady state ----------------------------------------------
        pending = None
        for i, (hp, nn) in enumerate(blocks):
            need = DEADLINE.get(i, 0)
            while pulled[0] < need:
                pull(1)
            nxt = blocks[i + 1] if i + 1 < len(blocks) else None
            nxt_pts = []
            # finish the PREVIOUS block's normalize first: its recips are
            # long done, so the broadcasts + mul overlap this block's AV
            # instead of queueing behind this block's reciprocal chain.
            if pending is not None:
                div_finish(pending)
                pending = None
            if i == 11:
                # all nn0 oT slabs are normalized; the output projection
                # becomes this block's filler so it overlaps the final AV.
                filler_gen[0] = gen_e_block(0)
            av0 = psp.tile([128, 512], f32, tag="av", bufs=2, name="av0")
            av1 = psp.tile([128, 512], f32, tag="av", bufs=2, name="av1")
            for mt in range(MT):
                av_step(av0, hp, 0, mt, cur_pts)
                av_step(av1, hp, 1, mt, cur_pts)
                if nxt:
                    s_block_step(nxt[0], nxt[1], mt, nxt_pts)
                pull(2)
            st_div = div_start(hp, nn, (av0, av1))
            pull(2)
            pending = st_div
            cur_pts = nxt_pts
        for _ in filler_gen[0]:   # drain any remaining e_block(0) work
            pass
        div_finish(pending)
        for _ in gen_e_block(1):
            pass

        psp.release()
        data.release()

    # Keep semaphore waits on the MATMULs instead of migrating them onto
    # their LDWEIGHTS: a wait-carrying LDWEIGHTS cannot be pulled ahead by
    # the PE's reorder window, which costs ~100ns per affected matmul.
    # Extra waits get split into EVENT_SEMAPHORE instructions instead.
    nc.move_matmul_waits_to_ldweights = lambda: None
    nc.compile()
    return nc


def _get_nc():
    if "nc" not in _CACHE:
        _CACHE["nc"] = _build_nc()
    return _CACHE["nc"]


def _prep_in_maps(x, qkv_w, proj_w, proj_b):
    import ml_dtypes

    bf16 = ml_dtypes.bfloat16
    x = np.asarray(x, dtype=np.float32)
    qkv_w = np.asarray(qkv_w, dtype=np.float32)
    proj_w = np.asarray(proj_w, dtype=np.float32)
    proj_b = np.asarray(proj_b, dtype=np.float32)

    def by_hp(wT):  # [C(in), C(out)] -> sbuf layout [128, HP*CT*128]
        w4 = wT.reshape(CT, 128, HP, 128).transpose(1, 2, 0, 3)
        return np.ascontiguousarray(w4.reshape(128, HP * CT * 128)).astype(bf16)

    def by_ci(wT):  # [C(in), C(out)] -> sbuf layout [128, CT*C]
        w3 = wT.reshape(CT, 128, C).transpose(1, 0, 2)
        return np.ascontiguousarray(w3.reshape(128, CT * C)).astype(bf16)

    wq_hp = by_hp(np.ascontiguousarray(qkv_w[0:C].T))         # [in, out] tiled
    wk_hp = by_hp(np.ascontiguousarray(qkv_w[C:2 * C].T))
    wvT = by_ci(np.ascontiguousarray(qkv_w[2 * C:3 * C].T))
    pwT = by_ci(np.ascontiguousarray(proj_w.T))
    pb = np.ascontiguousarray(proj_b.reshape(CT, 128).T)      # [128, CT] f32
    ones2 = np.zeros((2, 128), dtype=np.float32)
    ones2[0, 0:64] = 1.0
    ones2[1, 64:128] = 1.0
    ones2 = ones2.astype(bf16)

    in_maps = []
    for b in range(B):
        # xT sbuf layout [128, CT*N]: col ci*N+n = x[n, ci*128+p]
        xt = np.ascontiguousarray(
            x[b].T.reshape(CT, 128, N).transpose(1, 0, 2).reshape(128, CT * N)
        ).astype(bf16)
        in_maps.append({
            "xT": xt,
            "wqT": wq_hp, "wkT": wk_hp, "wvT": wvT, "pwT": pwT, "pb": pb,
            "ones2": ones2,
        })
    return in_maps


def _run(in_maps, **kwargs):
    from concourse.bass_utils import run_bass_kernel_spmd

    return run_bass_kernel_spmd(_get_nc(), in_maps,
                                core_ids=list(range(NCORES)), **kwargs)


def _gather(res):
    out = np.stack([res.results[b]["out"].T for b in range(B)], axis=0)
    return np.ascontiguousarray(out.astype(np.float32))


def kernel(x, qkv_w, proj_w, proj_b):
    return _gather(_run(_prep_in_maps(x, qkv_w, proj_w, proj_b)))
